# revision 5
# baseline (speedup 1.0000x reference)
"""Bottleneck-attention (BoTNet-style) kernel for Trainium2, 8 NeuronCores.

Data-parallel over batch: core b computes batch element b entirely locally
(no collectives). Returns (out, sim) like the reference.

Per-core computation (b=1, c=512, heads=8, d=64, 32x32 pixels):
  qkv   = w_qkv @ fmap           (1x1 conv as channel matmul, f32r)
  sim   = q^T k * scale + pos    (augmented matmul: contraction dim 128 =
                                  64 q-dims + 32 rel-W rows + 32 rel-H rows
                                  against k / Jsel / Xsel selection consts)
  attn  = softmax(sim)           (exp on ScalarE w/ fused row-sum accum)
  out   = attn @ v               (bf16, attn transposed via TensorE)
"""
import sys

sys.path.insert(0, "/opt/trn_rl_repo")

from contextlib import ExitStack

import numpy as np
import ml_dtypes

import concourse.bass as bass
import concourse.mybir as mybir
import concourse.tile as tile
from concourse import bacc
from concourse.bass_utils import run_bass_kernel_spmd

HEADS = 8
C = 512
D = 64  # dim per head
HW = 1024  # 32*32 pixels
W = 32
SCALE = D ** -0.5

F32 = mybir.dt.float32
F32R = mybir.dt.float32r
BF16 = mybir.dt.bfloat16

_CACHED_NC = None


def build_nc():
    nc = bacc.Bacc("TRN2", target_bir_lowering=False, debug=False)

    fmap_d = nc.declare_dram_parameter("fmap", [C, HW], F32R, isOutput=False)
    wt_d = nc.declare_dram_parameter("wt", [C, 3 * C], F32R, isOutput=False)
    relw_d = nc.declare_dram_parameter("relw", [128, HW], BF16, isOutput=False)
    relh_d = nc.declare_dram_parameter("relh", [128, HW], BF16, isOutput=False)
    jx_d = nc.declare_dram_parameter("jx", [64, HW], F32R, isOutput=False)
    ident_d = nc.declare_dram_parameter("ident", [128, 128], BF16, isOutput=False)

    out_d = nc.declare_dram_parameter("out", [C, HW], F32, isOutput=True)
    sim_d = nc.declare_dram_parameter("sim", [HEADS, HW, HW], F32, isOutput=True)

    with tile.TileContext(nc) as tc, ExitStack() as ctx:
        # ---- persistent SBUF pools ----
        const_pool = ctx.enter_context(tc.tile_pool(name="const", bufs=1))
        big_pool = ctx.enter_context(tc.tile_pool(name="big", bufs=1))

        relw_sb = const_pool.tile([128, HW], BF16, tag="relw")
        relh_sb = const_pool.tile([128, HW], BF16, tag="relh")
        ident_sb = const_pool.tile([128, 128], BF16, tag="ident")
        nc.sync.dma_start(out=relw_sb, in_=relw_d[:, :])
        nc.sync.dma_start(out=relh_sb, in_=relh_d[:, :])
        nc.sync.dma_start(out=ident_sb, in_=ident_d[:, :])

        # AUG: stationary operand for the augmented sim matmul.
        #   per (head, tile) block of 128 cols (natural pixel order):
        #   even head: rows 0-63 q*scale, 64-95 RW^T, 96-127 RH^T
        #   odd  head: rows 0-31 RW^T, 32-63 RH^T, 64-127 q*scale
        aug = big_pool.tile([128, HEADS * HW], F32R, tag="aug")
        # KAUG: moving operand. rows pair with AUG rows:
        #   even head: rows 0-63 k, 64-95 Jsel, 96-127 Xsel
        #   odd  head: rows 0-31 Jsel, 32-63 Xsel, 64-127 k
        kaug = big_pool.tile([128, HEADS * HW], F32R, tag="kaug")
        # V in [pixel, channel] layout, bf16: [128, jchunk, 512]
        vsb = big_pool.tile([128, 8, C], BF16, tag="vsb")
        # scaled q in bf16 for the small rel-logit matmuls (f32r matmuls
        # cannot target PSUM partition base != 0; bf16 can)
        qbf = big_pool.tile([128, HEADS * HW], BF16, tag="qbf")

        for h in range(HEADS):
            if h % 2 == 0:
                nc.sync.dma_start(out=kaug[64:128, h * HW : (h + 1) * HW], in_=jx_d[:, :])
            else:
                nc.sync.dma_start(out=kaug[0:64, h * HW : (h + 1) * HW], in_=jx_d[:, :])

        # ---- phase 1: qkv matmuls + AUG/KAUG/VSB setup ----
        with tc.tile_pool(name="wf", bufs=1) as wf_pool:
            w_sb = wf_pool.tile([128, 4, 3 * C], F32R, tag="w")
            f_sb = wf_pool.tile([128, 4, HW], F32R, tag="f")
            nc.sync.dma_start(out=w_sb, in_=wt_d[:, :].rearrange("(kc p) o -> p kc o", p=128))
            nc.sync.dma_start(out=f_sb, in_=fmap_d[:, :].rearrange("(kc p) m -> p kc m", p=128))

            qkv_pools = ExitStack()
            ps_qk = qkv_pools.enter_context(tc.tile_pool(name="ps_qk", bufs=2, space="PSUM"))
            ps_v = qkv_pools.enter_context(tc.tile_pool(name="ps_v", bufs=2, space="PSUM"))
            # q o-tiles (heads 2m, 2m+1), scaled into AUG
            for m in range(4):
                ps = ps_qk.tile([128, HW], F32, tag="qk")
                for kc in range(4):
                    for nh in range(2):
                        nc.tensor.matmul(
                            ps[:, nh * 512 : (nh + 1) * 512],
                            w_sb[:, kc, m * 128 : (m + 1) * 128],
                            f_sb[:, kc, nh * 512 : (nh + 1) * 512],
                            start=(kc == 0),
                            stop=(kc == 3),
                        )
                he, ho = 2 * m, 2 * m + 1
                nc.scalar.mul(aug[0:64, he * HW : (he + 1) * HW], ps[0:64, :], SCALE)
                nc.scalar.mul(aug[64:128, ho * HW : (ho + 1) * HW], ps[64:128, :], SCALE)
                nc.vector.tensor_scalar_mul(qbf[0:64, he * HW : (he + 1) * HW], ps[0:64, :], SCALE)
                nc.vector.tensor_scalar_mul(qbf[64:128, ho * HW : (ho + 1) * HW], ps[64:128, :], SCALE)

            # k o-tiles into KAUG
            for m in range(4):
                ps = ps_qk.tile([128, HW], F32, tag="qk")
                for kc in range(4):
                    for nh in range(2):
                        nc.tensor.matmul(
                            ps[:, nh * 512 : (nh + 1) * 512],
                            w_sb[:, kc, C + m * 128 : C + (m + 1) * 128],
                            f_sb[:, kc, nh * 512 : (nh + 1) * 512],
                            start=(kc == 0),
                            stop=(kc == 3),
                        )
                he, ho = 2 * m, 2 * m + 1
                nc.vector.tensor_copy(kaug[0:64, he * HW : (he + 1) * HW], ps[0:64, :])
                nc.vector.tensor_copy(kaug[64:128, ho * HW : (ho + 1) * HW], ps[64:128, :])

            # v p-tiles into VSB (pixel-major, bf16)
            for pt in range(8):
                ps = ps_v.tile([128, C], F32, tag="v")
                for kc in range(4):
                    nc.tensor.matmul(
                        ps[:, :],
                        f_sb[:, kc, pt * 128 : (pt + 1) * 128],
                        w_sb[:, kc, 2 * C : 3 * C],
                        start=(kc == 0),
                        stop=(kc == 3),
                    )
                nc.vector.tensor_copy(vsb[:, pt, :], ps[:, :])

            qkv_pools.close()
            rw_pools = ExitStack()
            ps_rw = rw_pools.enter_context(tc.tile_pool(name="ps_rw", bufs=1, space="PSUM"))
            # relative-logit rows. FP32-class matmul outputs may only land at
            # PSUM partition base 0 (col_grp 0x3) or 64 (0xc), so RW and RH
            # get separate tiles (even head at 64, odd at 0); the evacuation
            # copies shift RH into AUG rows 96-127 / 32-63.
            for m in range(4):
                he, ho = 2 * m, 2 * m + 1
                ps_a = ps_rw.tile([128, HW], F32, tag="rw")  # RW^T
                ps_b = ps_rw.tile([128, HW], F32, tag="rh")  # RH^T
                for h, qrows, pbase in ((he, slice(0, 64), 64), (ho, slice(64, 128), 0)):
                    qh = qbf[qrows, h * HW : (h + 1) * HW]
                    q_yx = qh.rearrange("p (x y) -> p y x", y=W)  # [64, y, x]
                    q_xy = qh.rearrange("p (x y) -> p x y", y=W)  # [64, x, y]
                    for y in range(W):
                        nc.tensor.matmul(
                            ps_a[pbase : pbase + 32, y * W : (y + 1) * W],
                            relw_sb[qrows, y * W : (y + 1) * W],
                            q_yx[:, y, :],
                            start=True,
                            stop=True,
                            tile_position=(qrows.start, pbase),
                        )
                    for x in range(W):
                        nc.tensor.matmul(
                            ps_b[pbase : pbase + 32, x * W : (x + 1) * W],
                            relh_sb[qrows, x * W : (x + 1) * W],
                            q_xy[:, x, :],
                            start=True,
                            stop=True,
                            tile_position=(qrows.start, pbase),
                        )
                # evacuate: RW rows need (y,x)->(x,y) column permute; RH rows
                # straight but shifted +32 partitions.
                for h, pbase, rw_base, rh_base in ((he, 64, 64, 96), (ho, 0, 0, 32)):
                    dst_rw = aug[rw_base : rw_base + 32, h * HW : (h + 1) * HW]
                    src_rw = ps_a[pbase : pbase + 32, :].rearrange("p (y x) -> p x y", x=W)
                    nc.vector.tensor_copy(dst_rw.rearrange("p (x y) -> p x y", y=W), src_rw)
                    dst_rh = aug[rh_base : rh_base + 32, h * HW : (h + 1) * HW]
                    nc.vector.tensor_copy(dst_rh, ps_b[pbase : pbase + 32, :])

            rw_pools.close()

        # ---- phase 2: attention per head ----
        with (
            tc.tile_pool(name="work", bufs=2) as work_pool,
            tc.tile_pool(name="stage", bufs=3) as stage_pool,
            tc.tile_pool(name="expT", bufs=2) as expT_pool,
            tc.tile_pool(name="small", bufs=2) as small_pool,
            tc.tile_pool(name="ps_sim", bufs=2, space="PSUM") as ps_sim,
            tc.tile_pool(name="ps_t", bufs=2, space="PSUM") as ps_t,
            tc.tile_pool(name="ps_o", bufs=1, space="PSUM") as ps_o,
        ):
            for h in range(HEADS):
                rowsum = small_pool.tile([128, 8], F32, tag="rowsum")
                recip = small_pool.tile([128, 8], F32, tag="recip")
                expT_h = expT_pool.tile([128, 8 * HW], BF16, tag="expT")
                for t in range(8):
                    bi = h * 8 + t
                    ps_s = ps_sim.tile([128, HW], F32, tag="sim")
                    for nh in range(2):
                        nc.tensor.matmul(
                            ps_s[:, nh * 512 : (nh + 1) * 512],
                            aug[:, bi * 128 : (bi + 1) * 128],
                            kaug[:, h * HW + nh * 512 : h * HW + (nh + 1) * 512],
                            start=True,
                            stop=True,
                        )
                    # exp with fused row-sum; sim evacuation for DRAM output
                    exp_sb = work_pool.tile([128, HW], BF16, tag="exp")
                    nc.scalar.activation(
                        out=exp_sb[:, :],
                        in_=ps_s[:, :],
                        func=mybir.ActivationFunctionType.Exp,
                        accum_out=rowsum[:, t : t + 1],
                    )
                    sim_stage = stage_pool.tile([128, HW], F32, tag="simst")
                    if t % 2 == 0:
                        nc.vector.tensor_copy(sim_stage[:, :], ps_s[:, :])
                    else:
                        nc.scalar.copy(sim_stage[:, :], ps_s[:, :])
                    nc.sync.dma_start(
                        out=sim_d[h, t * 128 : (t + 1) * 128, :], in_=sim_stage[:, :]
                    )
                    # normalize (bf16 4x) and transpose
                    nc.vector.reciprocal(recip[:, t : t + 1], rowsum[:, t : t + 1])
                    expn_sb = work_pool.tile([128, HW], BF16, tag="expn")
                    nc.vector.tensor_scalar_mul(expn_sb[:, :], exp_sb[:, :], recip[:, t : t + 1])
                    ps_tr = ps_t.tile([128, HW], BF16, tag="tr")
                    for cc in range(8):
                        nc.tensor.transpose(
                            ps_tr[:, cc * 128 : (cc + 1) * 128],
                            expn_sb[:, cc * 128 : (cc + 1) * 128],
                            ident_sb[:, :],
                        )
                    nc.vector.tensor_copy(
                        expT_h.rearrange("p (c i) -> p c i", c=8)[:, :, t * 128 : (t + 1) * 128],
                        ps_tr.rearrange("p (c i) -> p c i", c=8),
                    )
                # attn @ v (accumulate over j-chunks)
                ps_out = ps_o.tile([64, HW], F32, tag="out")
                for jc in range(8):
                    for nh in range(2):
                        nc.tensor.matmul(
                            ps_out[:, nh * 512 : (nh + 1) * 512],
                            vsb[:, jc, h * D : (h + 1) * D],
                            expT_h[:, jc * HW + nh * 512 : jc * HW + (nh + 1) * 512],
                            start=(jc == 0),
                            stop=(jc == 7),
                        )
                out_sb = stage_pool.tile([64, HW], F32, tag="outst")
                nc.scalar.copy(out_sb[:, :], ps_out[:, :])
                nc.sync.dma_start(out=out_d[h * D : (h + 1) * D, :], in_=out_sb[:, :])

    nc.compile()
    return nc


def _expand_rel(rel):
    # rel: [63, 64] -> [128, 1024] where col y*32+j holds rel[j - y + 31, :]
    # (duplicated on both partition halves so either base partition works)
    j = np.arange(W)[None, :]
    y = np.arange(W)[:, None]
    m = j - y + (W - 1)  # [32, 32] in [0, 62]
    e = rel[m]  # [32, 32, 64] (y, j, d)
    e = e.transpose(2, 0, 1).reshape(D, W * W)  # [64, 1024]
    return np.concatenate([e, e], axis=0).astype(ml_dtypes.bfloat16)  # [128, 1024]


def _build_jx():
    eye = np.eye(W, dtype=np.float32)
    jsel = np.tile(eye, (1, W))  # [32, 1024]: block x2 = I
    xsel = np.repeat(eye, W, axis=1)  # [32, 1024]: col x2*32+j -> delta(x2', x2)
    return np.concatenate([jsel, xsel], axis=0)  # [64, 1024]


def _get_nc():
    global _CACHED_NC
    if _CACHED_NC is None:
        _CACHED_NC = build_nc()
    return _CACHED_NC


def kernel(fmap, w_qkv, rel_emb_w, rel_emb_h, _trace=False, _trace_kwargs=None):
    fmap = np.asarray(fmap, dtype=np.float32)
    w_qkv = np.asarray(w_qkv, dtype=np.float32)
    rel_emb_w = np.asarray(rel_emb_w, dtype=np.float32)
    rel_emb_h = np.asarray(rel_emb_h, dtype=np.float32)
    b = fmap.shape[0]

    nc = _get_nc()
    wt = np.ascontiguousarray(w_qkv.T)  # [512, 1536]
    relw = _expand_rel(rel_emb_w)
    relh = _expand_rel(rel_emb_h)
    jx = _build_jx()
    ident = np.eye(128, dtype=ml_dtypes.bfloat16)

    in_maps = []
    for i in range(b):
        in_maps.append(
            {
                "fmap": np.ascontiguousarray(fmap[i].reshape(C, HW)),
                "wt": wt,
                "relw": relw,
                "relh": relh,
                "jx": jx,
                "ident": ident,
            }
        )
    kwargs = {}
    if _trace:
        kwargs["trace"] = True
        if _trace_kwargs:
            kwargs.update(_trace_kwargs)
    res = run_bass_kernel_spmd(nc, in_maps, core_ids=list(range(b)), **kwargs)
    out = np.stack([res.results[i]["out"] for i in range(b)]).reshape(b, C, W, W)
    sim = np.stack([res.results[i]["sim"] for i in range(b)])
    kernel.last_results = res
    return out, sim
